# revision 15
# baseline (speedup 1.0000x reference)
"""Trainium2 Bass kernel for batched multi-head self-attention.

Reference computation (per batch element b):
    qkv = x @ w_qkv.T                  # [N, 3C]
    q, k, v = split/reshape to heads   # H=16 heads, d=64
    attn = softmax(q @ k.T / sqrt(d))
    out = (attn @ v) reshaped back     # [N, C]
    y = out @ w_proj.T + b_proj

Sharding: pure data-parallel over batch B=8 across the 8 NeuronCores
(one batch element per core, weights replicated, no collectives).

Key structure (vs the 229us all-bf16 version):
  - TimelineSim matmul cost = out_free_size x cycles_per_row; fp8e4
    DoubleRow runs 0.5 cycles/row with K=256 lanes -> 4x bf16
    throughput per K-column.
  - qkv projection in compensated fp8: x = x_hi+x_lo, w = w_hi+w_lo
    (exact e4m3 splits, host-side; w prescaled 32x against e4m3
    subnormals), products hi*hi + lo*hi + hi*lo -> ~0.1% operand
    error at 12 DoubleRow chunks per [128,512] tile (vs 16 bf16).
  - QK^T is the exact 4-term (q_hi+q_lo)(k_hi+k_lo) in ONE K=256
    DoubleRow instr per [128m,512n] score tile: rhs = [q_hi;q_lo]
    (stride-0 broadcast middle dim), stationary = [[k_hi;k_hi],
    [k_lo;k_lo]] built by dup-DMAs. q_hi/q_lo written straight into
    per-head stacks with partition-base-shifted DVE writes.
  - P stays bf16 (raw-fp8 P measured 2.1% end-to-end > 2e-2 gate), so
    attn@V and out-proj stay bf16. Row sums via a 32.0-valued extra V
    column (un-does the 32x w prescale at normalization for free).
  - exp on ACT as [128,1024] ops spanning 2 PSUM banks (QK fills the
    two halves) -> 128 x 1038ns = 133us, under the PE's ~171us.
  - Emission is n2-outer with a filler queue: the ACT-bound attention
    stream (640ns PE vs 1038ns ACT per exp unit) is padded with
    next-pair projection chunks and (in the n2=1 half) out-projection
    passes, so the PE never starves while ACT grinds exps. AV is
    software-pipelined one unit behind QK; av PSUM banks release via
    an immediate SBUF evacuation, with normalize reading the copy.
  - rel err ~4.2e-3 measured on hardware vs the 2e-2 gate.
"""

import os
import sys
from collections import deque

for _p in ("/opt/trn_rl_repo", "/root/.axon_site/_ro/trn_rl_repo"):
    if os.path.isdir(_p) and _p not in sys.path:
        sys.path.insert(0, _p)
        break

import numpy as np
import ml_dtypes

import concourse.bass as bass
import concourse.bacc as bacc
import concourse.tile as tile
import concourse.mybir as mybir
from concourse import bass_utils

BF16 = mybir.dt.bfloat16
F32 = mybir.dt.float32
FP8 = mybir.dt.float8e4
AF = mybir.ActivationFunctionType
DR = mybir.MatmulPerfMode.DoubleRow
E4 = ml_dtypes.float8_e4m3

B, N, C, H = 8, 1024, 1024, 16
D = C // H            # 64 head dim
P = 128               # partitions
PAIRS = H // 2
NT2 = 2               # two 512-wide n tiles
WSCALE = 32.0
EXPSCALE = (float(D) ** -0.5) / (WSCALE * WSCALE)
N_CORES = 8

_cache = {}


def _build():
    nc = bacc.Bacc("TRN2", target_bir_lowering=False, debug=False,
                   enable_asserts=False, num_devices=N_CORES)

    xh_d = nc.dram_tensor("xh", [P, 8 * N], FP8, kind="ExternalInput")
    xl_d = nc.dram_tensor("xl", [P, 8 * N], FP8, kind="ExternalInput")
    wh_d = nc.dram_tensor("wh", [P, 8 * 3 * C], FP8, kind="ExternalInput")
    wl_d = nc.dram_tensor("wl", [P, 8 * 3 * C], FP8, kind="ExternalInput")
    wp_d = nc.dram_tensor("wp", [P, 8 * C], BF16, kind="ExternalInput")
    bias_d = nc.dram_tensor("bias", [P, 8], F32, kind="ExternalInput")
    outT_d = nc.dram_tensor("outT", [C, N], BF16, kind="ExternalOutput")

    with tile.TileContext(nc) as tc:
        with (
            tc.tile_pool(name="res", bufs=1) as rp,
            tc.tile_pool(name="work", bufs=2) as wp,
            tc.tile_pool(name="ps", bufs=1, space="PSUM") as pp,
        ):
            # ---------------- PE warm-up ----------------
            warm_a = wp.tile([P, 512], BF16, name="warm_a", tag="warm_a",
                             bufs=1)
            nc.gpsimd.memset(warm_a[:], 0.25)
            warm_ps = pp.tile([P, 512], F32, name="warm_ps", tag="acc",
                              bufs=2)
            for _ in range(12):
                nc.tensor.matmul(warm_ps[:], warm_a[:, 0:P], warm_a[:],
                                 start=True, stop=True)

            # ---------------- resident inputs ----------------
            xh = rp.tile([P, 8, N], FP8, name="xh", tag="xh")
            xl = rp.tile([P, 8, N], FP8, name="xl", tag="xl")
            wh = rp.tile([P, 8, 3 * C], FP8, name="wh", tag="wh")
            wl = rp.tile([P, 8, 3 * C], FP8, name="wl", tag="wl")
            wpj = rp.tile([P, 8, C], BF16, name="wpj", tag="wpj")
            bias_t = rp.tile([P, 8], F32, name="bias_t", tag="bias")

            def ld(dst, src_d, lo, hi, c0, c1):
                nc.sync.dma_start(
                    dst[:, c0:c1, lo:hi],
                    src_d.ap().rearrange("p (c n) -> p c n",
                                         c=8)[:, c0:c1, lo:hi])

            # v-projection operands first, split by cj-half so the first
            # chunks can start after ~2 small DMAs (chunk j uses cj-halves
            # [0:4] for j%4<2, [4:8] for j%4>=2)
            for c0, c1 in ((0, 4), (4, 8)):
                ld(xh, xh_d, 0, N, c0, c1)
                ld(wh, wh_d, 2 * C, 3 * C, c0, c1)
                ld(xl, xl_d, 0, N, c0, c1)
                ld(wl, wl_d, 2 * C, 3 * C, c0, c1)
            for c0, c1 in ((0, 4), (4, 8)):
                ld(wh, wh_d, 0, 2 * C, c0, c1)     # q,k cols
                ld(wl, wl_d, 0, 2 * C, c0, c1)
            nc.sync.dma_start(bias_t[:], bias_d.ap())
            # wpj is loaded later (only the output projection needs it)

            # ---------------- result tiles ----------------
            vt = [[rp.tile([P, 8, D + 1], BF16, name=f"v{m}_{j}",
                           tag=f"v{m}_{j}") for j in range(2)]
                  for m in range(8)]
            for m in range(8):
                for j in range(2):
                    nc.vector.memset(vt[m][j][:, :, D:D + 1], WSCALE)

            qh_t = [rp.tile([P, N], FP8, name=f"qh{h}", tag=f"qh{h}")
                    for h in range(H)]
            kh_t = [rp.tile([P, 2, N], FP8, name=f"kh{h}", tag=f"kh{h}")
                    for h in range(H)]
            ao = [rp.tile([P, N], BF16, name=f"ao{i}", tag=f"ao{i}")
                  for i in range(PAIRS)]

            # ---------------- filler machinery ----------------
            # two priorities: "hi" (qk-projection chains, deadline-bound)
            # drains before "lo" (out-projection passes, anytime work)
            hi_q = deque()
            lo_q = deque()

            def pump(budget):
                spent = 0.0
                while spent < budget:
                    q = hi_q if hi_q else lo_q
                    if not q:
                        return
                    try:
                        spent += next(q[0])
                    except StopIteration:
                        q.popleft()

            def drain_hi():
                while hi_q:
                    try:
                        next(hi_q[0])
                    except StopIteration:
                        hi_q.popleft()

            def drain_all():
                drain_hi()
                while lo_q:
                    try:
                        next(lo_q[0])
                    except StopIteration:
                        lo_q.popleft()

            # ---------------- stage-A chunk helper ----------------
            def a_chunk(ps, osl, nsl, which, j, start, stop):
                cj = 2 * (j % 4)
                xop = xl if 4 <= j < 8 else xh
                wop = wl if j >= 8 else wh
                if which == "qk":
                    lhsT = wop[:, cj:cj + 2, osl]
                    rhs = xop[:, cj:cj + 2, nsl]
                else:
                    lhsT = xop[:, cj:cj + 2, nsl]
                    rhs = wop[:, cj:cj + 2, osl]
                nc.tensor.matmul(ps[:], lhsT, rhs, start=start, stop=stop,
                                 perf_mode=DR)

            # ---------------- chain generators ----------------
            def q_chain(pr, n2):
                """q projection for pair pr, one n2 half; quantizes into
                the per-head [q_hi; q_lo] stacks (shifted DVE writes)."""
                h0, h1 = 2 * pr, 2 * pr + 1
                nsl = slice(n2 * 512, (n2 + 1) * 512)
                ps = pp.tile([P, 512], F32, name=f"accq{pr}_{n2}", tag="acc",
                             bufs=2)
                for j in range(12):
                    a_chunk(ps, slice(pr * P, (pr + 1) * P), nsl, "qk", j,
                            j == 0, j == 11)
                    yield 107.0
                nc.vector.tensor_copy(qh_t[h0][0:D, nsl], ps[0:D, :])
                nc.vector.tensor_copy(qh_t[h1][0:D, nsl], ps[D:P, :])
                nc.vector.tensor_sub(qh_t[h0][D:P, nsl], ps[0:D, :],
                                     qh_t[h0][0:D, nsl])
                nc.vector.tensor_sub(qh_t[h1][D:P, nsl], ps[D:P, :],
                                     qh_t[h1][0:D, nsl])

            def k_chain(pr, kscr, half):
                """k projection for pair pr, one m half into kscr; on the
                second half, dup-DMA the per-head KH stacks."""
                h0, h1 = 2 * pr, 2 * pr + 1
                msl = slice(half * 512, (half + 1) * 512)
                ps = pp.tile([P, 512], F32, name=f"acck{pr}_{half}",
                             tag="acc", bufs=2)
                for j in range(12):
                    a_chunk(ps, slice(C + pr * P, C + (pr + 1) * P), msl,
                            "qk", j, j == 0, j == 11)
                    yield 107.0
                nc.vector.tensor_copy(kscr[:, 0, msl], ps[:])
                nc.vector.tensor_sub(kscr[:, 1, msl], ps[:], kscr[:, 0, msl])
                if half == 1:
                    for hi_, h in ((0, h0), (1, h1)):
                        src = kscr[hi_ * D:(hi_ + 1) * D, :, :]
                        nc.sync.dma_start(kh_t[h][0:D, :, :], src)
                        nc.sync.dma_start(kh_t[h][D:P, :, :], src)

            def v_chain(m, j):
                ps = pp.tile([P, 512], F32, name=f"accv{m}_{j}",
                             tag=("acc", "av")[(2 * m + j) % 2], bufs=2)
                for jj in range(12):
                    a_chunk(ps, slice(2 * C + 512 * j, 2 * C + 512 * (j + 1)),
                            slice(m * P, (m + 1) * P), "v", jj,
                            jj == 0, jj == 11)
                    yield 107.0
                nc.vector.tensor_copy(
                    vt[m][j][:, :, 0:D],
                    ps[:].rearrange("p (h d) -> p h d", d=D))

            def proj_chain(ot, n2):
                nsl = slice(n2 * 512, (n2 + 1) * 512)
                ps = pp.tile([P, 512], F32, name=f"accy{ot}_{n2}", tag="acc",
                             bufs=2)
                for pr in range(PAIRS):
                    nc.tensor.matmul(ps[:], wpj[:, pr, ot * P:(ot + 1) * P],
                                     ao[pr][:, nsl], start=(pr == 0),
                                     stop=(pr == PAIRS - 1))
                    yield 213.0
                yt = wp.tile([P, 512], BF16, name=f"y{ot}_{n2}", tag="y",
                             bufs=3)
                if n2 == 1:
                    # tail chains: ACT is idle after the last exp
                    nc.scalar.activation(yt[:], ps[:], AF.Identity,
                                         bias=bias_t[:, ot:ot + 1], scale=1.0)
                else:
                    nc.vector.tensor_scalar_add(yt[:], ps[:],
                                                bias_t[:, ot:ot + 1])
                nc.sync.dma_start(outT_d.ap()[ot * P:(ot + 1) * P, nsl],
                                  yt[:])

            # ---------------- attention units ----------------
            def attention_section(pr, n2):
                """8 exp-units for pair pr, one n2 half. AV is pipelined one
                unit behind QK; fillers pumped between instrs."""
                h0, h1 = 2 * pr, 2 * pr + 1
                nsl = slice(n2 * 512, (n2 + 1) * 512)
                pending = None   # () -> emits delayed AV pair + maybe evac
                avt = {}
                for hi_, h in ((0, h0), (1, h1)):
                    avt[hi_] = pp.tile([D + 1, 512], F32,
                                       name=f"av{pr}_{n2}_{hi_}", tag="av",
                                       bufs=2)
                    qrhs = qh_t[h][:, nsl].unsqueeze(1).broadcast_to(
                        [P, 2, 512])
                    for mp in range(4):
                        stw = pp.tile([P, 1024], F32,
                                      name=f"st{pr}_{n2}_{hi_}_{mp}",
                                      tag="st", bufs=2)
                        for half in range(2):
                            m = 2 * mp + half
                            nc.tensor.matmul(
                                stw[:, half * 512:(half + 1) * 512],
                                kh_t[h][:, :, m * P:(m + 1) * P],
                                qrhs, start=True, stop=True, perf_mode=DR)
                        pw = wp.tile([P, 1024], BF16,
                                     name=f"pw{pr}_{n2}_{hi_}_{mp}",
                                     tag="pw", bufs=3)
                        nc.scalar.activation(pw[:], stw[:], AF.Exp,
                                             scale=EXPSCALE)
                        pump(210)
                        if pending is not None:
                            pending()
                        pump(210)

                        def make_av(hi_=hi_, h=h, mp=mp, pw=pw):
                            def emit():
                                for half in range(2):
                                    m = 2 * mp + half
                                    nc.tensor.matmul(
                                        avt[hi_][:],
                                        vt[m][h // 8][:, h % 8, :],
                                        pw[:, half * 512:(half + 1) * 512],
                                        start=(mp == 0 and half == 0),
                                        stop=(mp == 3 and half == 1))
                                if mp == 3:
                                    _finish_head(pr, n2, hi_, h, avt[hi_],
                                                 nsl)
                            return emit
                        pending = make_av()
                if pending is not None:
                    pending()

            def _finish_head(pr, n2, hi_, h, av, nsl):
                # evacuate av psum fast (frees the bank), then normalize
                # from the SBUF copy: aon = araw[0:64] / araw[64].
                # For the final section the bank release doesn't matter;
                # read the psum directly to shorten the tail chain.
                last = (pr == PAIRS - 1 and n2 == NT2 - 1)
                if last:
                    araw = av
                else:
                    araw = wp.tile([D + 1, 512], F32,
                                   name=f"ar{pr}_{n2}_{hi_}", tag="araw",
                                   bufs=4)
                    nc.vector.tensor_copy(araw[:], av[:])
                r0 = wp.tile([1, 512], F32, name=f"r0_{pr}_{n2}_{hi_}",
                             tag="r0", bufs=4)
                nc.vector.reciprocal(r0[:], araw[D:D + 1, :])
                bc = wp.tile([D, 512], F32, name=f"bc{pr}_{n2}_{hi_}",
                             tag="bc", bufs=4)
                nc.gpsimd.partition_broadcast(bc[:], r0[:])
                # even head: Pool (aligned, SBUF-only) unless reading psum;
                # odd head: DVE (proven partition-base-shifted write)
                eng = nc.gpsimd if (hi_ == 0 and not last) else nc.vector
                eng.tensor_mul(ao[pr][hi_ * D:(hi_ + 1) * D, nsl],
                               araw[0:D, :], bc[:])

            # ================= emission =================
            # prologue: v projection + pair-0 q/k chains
            for m in range(8):
                for j in range(2):
                    hi_q.append(v_chain(m, j))
            kscr = {}
            for pr in range(PAIRS):
                kscr[pr] = wp.tile([P, 2, N], FP8, name=f"kscr{pr}",
                                   tag="kscr", bufs=2)
            hi_q.append(k_chain(0, kscr[0], 0))
            hi_q.append(k_chain(0, kscr[0], 1))
            hi_q.append(q_chain(0, 0))
            drain_hi()

            # n2-outer attention with fillers. Before each section, the
            # previous enqueues (that section's own operands) must have
            # fully emitted: drain_hi() is the stale-read barrier.
            for n2 in range(NT2):
                for pr in range(PAIRS):
                    if n2 == 0 and pr == 3:
                        # wpj load deferred: keeps the DMA pipe clear for
                        # the early k-side dup DMAs
                        nc.sync.dma_start(
                            wpj[:], wp_d.ap().rearrange("p (c n) -> p c n",
                                                        c=8))
                    if n2 == 0:
                        if pr + 1 < PAIRS:
                            hi_q.append(k_chain(pr + 1, kscr[pr + 1], 0))
                            hi_q.append(k_chain(pr + 1, kscr[pr + 1], 1))
                            hi_q.append(q_chain(pr + 1, 0))
                        else:
                            hi_q.append(q_chain(0, 1))
                    else:
                        if pr + 1 < PAIRS:
                            hi_q.append(q_chain(pr + 1, 1))
                    attention_section(pr, n2)
                    # barrier: the next section's q/k stacks must be fully
                    # emitted before that section's QK reads them
                    drain_hi()
                    if n2 == 1 and pr == 0:
                        # ao[*][:, 0:512] all complete: queue proj(n2=0)
                        for ot in range(8):
                            lo_q.append(proj_chain(ot, 0))

            # tail: leftover fillers + proj(n2=1)
            for ot in range(8):
                lo_q.append(proj_chain(ot, 1))
            drain_all()

    nc.compile()
    return nc


def get_nc():
    if "nc" not in _cache:
        _cache["nc"] = _build()
    return _cache["nc"]


def _prep_host(x, w_qkv, w_proj, b_proj):
    xT = np.ascontiguousarray(x.T).astype(np.float32)        # [C, N]
    x_hi = xT.astype(E4)
    x_lo = (xT - x_hi.astype(np.float32)).astype(E4)
    wT = np.ascontiguousarray(w_qkv.T).astype(np.float32) * WSCALE
    w_hi = wT.astype(E4)
    w_lo = (wT - w_hi.astype(np.float32)).astype(E4)

    def lanes(a, cols):
        return np.ascontiguousarray(
            a.reshape(8, P, cols).transpose(1, 0, 2)).reshape(P, -1)

    wpT = np.ascontiguousarray(w_proj.T).astype(ml_dtypes.bfloat16)
    bias = np.ascontiguousarray(
        b_proj.reshape(8, P).T).astype(np.float32)
    return {
        "xh": lanes(x_hi, N), "xl": lanes(x_lo, N),
        "wh": lanes(w_hi, 3 * C), "wl": lanes(w_lo, 3 * C),
        "wp": lanes(wpT, C), "bias": bias,
    }


def kernel(x, w_qkv, w_proj, b_proj):
    x = np.asarray(x, dtype=np.float32)
    w_qkv = np.asarray(w_qkv, dtype=np.float32)
    w_proj = np.asarray(w_proj, dtype=np.float32)
    b_proj = np.asarray(b_proj, dtype=np.float32)

    shared = None
    in_maps = []
    for b in range(N_CORES):
        m = _prep_host(x[b], w_qkv, w_proj, b_proj)
        if shared is None:
            shared = {k: m[k] for k in ("wh", "wl", "wp", "bias")}
        m.update(shared)
        in_maps.append(m)

    nc = get_nc()
    _cache["in_maps"] = in_maps
    res = bass_utils.run_bass_kernel_spmd(nc, in_maps,
                                          core_ids=list(range(N_CORES)))
    out = np.empty((B, N, C), dtype=np.float32)
    for b in range(N_CORES):
        out[b] = res.results[b]["outT"].T.astype(np.float32)
    return out
